# revision 5
# baseline (speedup 1.0000x reference)
# Multi-head attention (b=2, n=2048, d_model=1024, 16 heads) on 8 NeuronCores.
#
# Sharding: core c = (batch b, head-group g) with b = c//4, g = c%4.
# Each core handles 1 batch element and 4 heads (256 channels), computing a
# partial output projection; the host sums the 4 group-partials per batch and
# adds b_O.
#
# Device layout (everything oriented so no transposes are needed):
#   xT   [D, N]      = x[b].T (bf16)             rhs of Q/K proj, lhsT of V
#   Q.T/K.T [2][128, N]  2 heads per 128-row tile (cs = h//2, rows (h%2)*64).
#   V    [N, CH] natural layout (+b_v), stored per-head with an appended
#        ones column: lhsT [m, 65] so the O-matmul's PSUM row 64 accumulates
#        the softmax denominators for free.
#   S.T  [m-slice, n] per head = K_h @ Q_h.T     (K=64 contraction)
#   E.T  = exp(S.T / 8) on ScalarE (scale folded into the activation), bf16
#   O.T+sums [65, n] = [V_h | 1].T @ E.T         (accumulate over m-slices)
#   Y.T  [D, N] = woT.T @ (O.T * recip(sums)), DVE-copied to fp16, DMA.
#
# Schedule: the attention loop (128 (chunk,head,ms) iterations) is paced by
# the ScalarE exp (~1.11us per [128,1024] tile).  All projection work is
# split into ~2-matmul "quanta" and drip-fed into the per-iteration slack by
# a deadline-driven scheduler, so the PE never idles and holds its 2.4 GHz
# p-state.  Softmax normalization is split: the reciprocal chain runs right
# after each head, but the (broadcast x multiply) into osb is deferred ~4
# iterations so the PE-queue broadcast matmul never stalls the stream.
#
# Matmul operands are bf16 (fp32 PSUM accumulation); fp32r measured ~3x
# slower on HW (cold-HAM equilibrium at ~630ns per 512-row matmul).

import ml_dtypes
import numpy as np

import concourse.bass as bass
import concourse.bacc as bacc
import concourse.tile as tile
from concourse import mybir
from concourse.bass_utils import run_bass_kernel_spmd

D = 1024  # d_model
N = 2048  # sequence length
B = 2  # batch
NHEADS = 16
DK = 64
NCORES = 8
GROUPS = 4  # head-groups across cores
HPG = NHEADS // GROUPS  # 4 heads per group
CH = HPG * DK  # 256 channels per group
KT = D // 128  # 8 contraction tiles for the projections
MS = N // 128  # 16 m-slices (key dim)
NCHUNK = 1024  # n-chunk width for the attention phase
NCHUNKS = N // NCHUNK

F32 = mybir.dt.float32
F16 = mybir.dt.float16
BF16 = mybir.dt.bfloat16


def _build_bass():
    nc = bacc.Bacc()

    xT_d = nc.dram_tensor("xT", [D, N], BF16, kind="ExternalInput")
    wqkv_d = nc.dram_tensor("wqkv", [D, 3 * CH], BF16, kind="ExternalInput")
    woT_d = nc.dram_tensor("woT", [CH, D], BF16, kind="ExternalInput")
    bq_d = nc.dram_tensor("bq", [CH], F32, kind="ExternalInput")
    bk_d = nc.dram_tensor("bk", [CH], F32, kind="ExternalInput")
    bv_d = nc.dram_tensor("bv", [CH], F32, kind="ExternalInput")
    yT_d = nc.dram_tensor("yT", [D, N], F16, kind="ExternalOutput")

    with tile.TileContext(nc) as tc:
        with (
            tc.tile_pool(name="persist", bufs=1) as persist,
            tc.tile_pool(name="ph1", bufs=1) as ph1,
            tc.tile_pool(name="et_pool", bufs=4) as et_pool,
            tc.tile_pool(name="osb_pool", bufs=2) as osb_pool,
            tc.tile_pool(name="small", bufs=2) as small,
            tc.tile_pool(name="aux_ps", bufs=2, space="PSUM") as aux_ps,
            tc.tile_pool(name="st_ps", bufs=2, space="PSUM") as st_pool,
            tc.tile_pool(name="ot_ps", bufs=1, space="PSUM") as ot_pool,
        ):
            # ---- persistent tensors ----
            qt = [persist.tile([128, N], BF16, tag=f"qt{cs}", name=f"qt{cs}") for cs in range(CH // 128)]
            kt = [persist.tile([128, N], BF16, tag=f"kt{cs}", name=f"kt{cs}") for cs in range(CH // 128)]
            v4 = [persist.tile([128, HPG * 65], BF16, tag=f"v4_{ms}", name=f"v4_{ms}") for ms in range(MS)]
            wot = [persist.tile([128, D], BF16, tag=f"wot{cs}", name=f"wot{cs}") for cs in range(CH // 128)]
            ones1 = persist.tile([1, 128], BF16, tag="ones1", name="ones1")

            # ---- input loads, spread across issue queues so posting isn't
            # ---- serialized on one engine.
            # gpsimd queue: constants + weights/biases not needed immediately
            nc.gpsimd.memset(ones1, 1.0)
            for ms in range(MS):
                v4v = v4[ms].rearrange("p (h c) -> p h c", c=65)
                nc.gpsimd.memset(v4v[:, :, 64:65], 1.0)
            for cs in range(CH // 128):
                nc.gpsimd.dma_start(out=wot[cs], in_=woT_d[cs * 128 : (cs + 1) * 128, :])
            bq_t, bk_t = [], []
            for bname, dram, lst in (("bq", bq_d, bq_t), ("bk", bk_d, bk_t)):
                for cs in range(CH // 128):
                    t = ph1.tile([128, 1], F32, tag=f"{bname}{cs}", name=f"{bname}{cs}")
                    nc.gpsimd.dma_start(out=t, in_=dram[cs * 128 : (cs + 1) * 128])
                    lst.append(t)
            bvb = ph1.tile([128, CH], F32, tag="bvb", name="bvb")
            bv_ap = bv_d[None, :]
            nc.gpsimd.dma_start(
                out=bvb,
                in_=bass.AP(tensor=bv_ap.tensor, offset=bv_ap.offset, ap=[[0, 128]] + list(bv_ap.ap[1:])),
            )
            # vector queue: packed qkv weights, k-tile order
            wq, wk, wv = [], [], []
            for k in range(KT):
                t = ph1.tile([128, 3 * CH], BF16, tag=f"wqkv{k}", name=f"wqkv{k}")
                nc.scalar.dma_start(out=t, in_=wqkv_d[k * 128 : (k + 1) * 128, :])
                wq.append(t[:, 0:CH])
                wk.append(t[:, CH : 2 * CH])
                wv.append(t[:, 2 * CH : 3 * CH])
            # sync queue: activations, first the columns the early chains need
            xt = [ph1.tile([128, N], BF16, tag=f"xt{k}", name=f"xt{k}") for k in range(KT)]
            for k in range(KT):
                nc.sync.dma_start(out=xt[k][:, 0:NCHUNK], in_=xT_d[k * 128 : (k + 1) * 128, 0:NCHUNK])
            for k in range(KT):
                nc.sync.dma_start(out=xt[k][:, NCHUNK:N], in_=xT_d[k * 128 : (k + 1) * 128, NCHUNK:N])

            osb_tiles = {}

            # ---- chain emitters, split into ~2-matmul quanta ----
            def qk_chain_quanta(isq, cs, n0):
                dst, w, bias = (qt, wq, bq_t) if isq else (kt, wk, bk_t)
                st_ = {}

                def step(k0, k1):
                    if k0 == 0:
                        st_["ps"] = aux_ps.tile([128, 512], F32, tag="aux", name="aux_ps_t")
                    ps = st_["ps"]
                    for k in range(k0, k1):
                        nc.tensor.matmul(
                            ps,
                            w[k][:, cs * 128 : (cs + 1) * 128],
                            xt[k][:, n0 : n0 + 512],
                            start=(k == 0),
                            stop=(k == KT - 1),
                        )
                    if k1 == KT:
                        nc.vector.tensor_scalar_add(
                            out=dst[cs][:, n0 : n0 + 512], in0=ps, scalar1=bias[cs]
                        )

                return [lambda a=a, b=b: step(a, b) for a, b in ((0, 2), (2, 4), (4, 6), (6, 8))]

            def v_chain_quanta(ms):
                st_ = {}

                def step(k0, k1):
                    if k0 == 0:
                        st_["ps"] = aux_ps.tile([128, 512], F32, tag="aux", name="aux_ps_t")
                    ps = st_["ps"]
                    for k in range(k0, k1):
                        nc.tensor.matmul(
                            ps[:, 0:CH],
                            xt[k][:, ms * 128 : (ms + 1) * 128],
                            wv[k],
                            start=(k == 0),
                            stop=(k == KT - 1),
                        )
                    if k1 == KT:
                        v4v = v4[ms].rearrange("p (h c) -> p h c", c=65)
                        nc.vector.tensor_add(
                            out=v4v[:, :, 0:64],
                            in0=ps[:, 0:CH].rearrange("p (h c) -> p h c", c=64),
                            in1=bvb.rearrange("p (h c) -> p h c", c=64),
                        )

                return [lambda a=a, b=b: step(a, b) for a, b in ((0, 2), (2, 4), (4, 6), (6, 8))]

            def emit_f(chunk, msl, j):
                n0 = chunk * NCHUNK
                osb = osb_tiles[chunk]
                yp = aux_ps.tile([128, 512], F32, tag="aux", name="aux_yt_t")
                for cs in range(CH // 128):
                    nc.tensor.matmul(
                        yp,
                        wot[cs][:, msl * 128 : (msl + 1) * 128],
                        osb[cs][:, j : j + 512],
                        start=(cs == 0),
                        stop=(cs == CH // 128 - 1),
                    )
                ysb = small.tile([128, 512], F16, tag="ysb", name="ysb_t", bufs=4)
                nc.vector.tensor_copy(out=ysb, in_=yp)
                nc.sync.dma_start(
                    out=yT_d[msl * 128 : (msl + 1) * 128, n0 + j : n0 + j + 512],
                    in_=ysb,
                )

            # deferred normalization part 2: broadcast recip over partitions
            # with a K=1 PE matmul, then scale O.T rows into osb.
            def norm2(chunk, h, oraw, rflat):
                cs, r0 = h // 2, (h % 2) * 64
                rb = st_pool.tile([128, NCHUNK], F32, tag="st", name="rb_t")
                for j in range(0, NCHUNK, 512):
                    nc.tensor.matmul(
                        rb[:, j : j + 512],
                        ones1[0:1, :],
                        rflat[0:1, j : j + 512],
                        start=True,
                        stop=True,
                    )
                nc.vector.tensor_mul(
                    out=osb_tiles[chunk][cs][r0 : r0 + 64, :],
                    in0=oraw[0:64, :],
                    in1=rb[0:64, :],
                )

            # ---- quantum schedule.  Chains are kept CONTIGUOUS in pop
            # order (only consecutive chains ever co-occupy the 2-buffer
            # aux PSUM pool); each chain gets a start deadline and its
            # quanta inherit start+qi.  Negative deadlines pop at iter 0.
            chains = []  # (start_deadline, ready_iter, [quanta...])
            for ms in range(1, MS):  # V(ms) must close before O at iter ms
                chains.append((ms - 4, 0, v_chain_quanta(ms)))
            for i, n0 in enumerate((512, 1024, 1536)):  # kt cs0, st @ iter 4i+4
                chains.append((4 * i + 1, 0, qk_chain_quanta(False, 0, n0)))
            for i, n0 in enumerate((0, 512)):  # qt cs1 chunk0 (h2c0 @ iter 32)
                chains.append((26 + 2 * i, 0, qk_chain_quanta(True, 1, n0)))
            for i, n0 in enumerate((0, 512, 1024, 1536)):  # kt cs1 (h2c0)
                rdy = 12 if n0 >= 1024 else 0
                chains.append((28 + 4 * i, rdy, qk_chain_quanta(False, 1, n0)))
            for i, n0 in enumerate((1024, 1536)):  # qt cs0 chunk1 (h0c1 @ 64)
                chains.append((52 + 4 * i, 12, qk_chain_quanta(True, 0, n0)))
            for i, n0 in enumerate((1024, 1536)):  # qt cs1 chunk1 (h2c1 @ 96)
                chains.append((84 + 4 * i, 12, qk_chain_quanta(True, 1, n0)))
            # output projection for chunk 0: fillers after norm2(c0,h3)@~67;
            # hold back the last 3 groups to cover the final reciprocal chain.
            fgroups = [(msl, j) for msl in range(D // 128) for j in range(0, NCHUNK, 512)]
            for i, (msl, j) in enumerate(fgroups[:13]):
                chains.append((70 + 4 * i, 69, [lambda m=msl, jj=j: emit_f(0, m, jj)]))
            tail_reserve = fgroups[13:]
            chains.sort(key=lambda c: c[0])
            quanta = [
                (start + qi, rdy, fn)
                for start, rdy, qs in chains
                for qi, fn in enumerate(qs)
            ]

            pending_norm2 = {}  # sched_iter -> callable

            def run_sched(t):
                if t in pending_norm2:
                    pending_norm2.pop(t)()
                popped = 0
                while quanta:
                    dl, rdy, fn = quanta[0]
                    due = dl <= t or any(q[0] <= t for q in quanta[1:6])
                    if due or (popped == 0 and rdy <= t):
                        quanta.pop(0)
                        fn()
                        popped += 1
                    else:
                        break

            # ---- prelude: what (h0, c0) iter-0 needs, in DMA-arrival order
            for fn in qk_chain_quanta(True, 0, 0):
                fn()
            for fn in qk_chain_quanta(True, 0, 512):
                fn()
            for fn in qk_chain_quanta(False, 0, 0):
                fn()
            for fn in v_chain_quanta(0):
                fn()

            # ---- attention + output projection ----
            it = 0
            for chunk in range(NCHUNKS):
                n0 = chunk * NCHUNK
                osb_tiles[chunk] = [
                    osb_pool.tile([128, NCHUNK], BF16, tag=f"osb{cs}", name=f"osb{cs}")
                    for cs in range(CH // 128)
                ]
                for h in range(HPG):
                    cs, r0 = h // 2, (h % 2) * 64
                    qt_h = qt[cs][r0 : r0 + 64, :]
                    kt_h = kt[cs][r0 : r0 + 64, :]
                    ot = ot_pool.tile([65, NCHUNK], F32, tag="ot", name="ot_t")
                    for ms in range(MS):
                        run_sched(it)
                        st = st_pool.tile([128, NCHUNK], F32, tag="st", name="st_t")
                        for j in range(0, NCHUNK, 512):
                            nc.tensor.matmul(
                                st[:, j : j + 512],
                                kt_h[:, ms * 128 : (ms + 1) * 128],
                                qt_h[:, n0 + j : n0 + j + 512],
                                start=True,
                                stop=True,
                            )
                        et = et_pool.tile([128, NCHUNK], BF16, tag="et", name="et_t")
                        nc.scalar.activation(
                            out=et,
                            in_=st,
                            func=mybir.ActivationFunctionType.Exp,
                            scale=float(1.0 / np.sqrt(DK)),
                        )
                        lhsT = v4[ms][:, h * 65 : (h + 1) * 65]
                        for j in range(0, NCHUNK, 512):
                            nc.tensor.matmul(
                                ot[:, j : j + 512],
                                lhsT,
                                et[:, j : j + 512],
                                start=(ms == 0),
                                stop=(ms == MS - 1),
                            )
                        it += 1
                    # normalization part 1: drain ot, reciprocal of row 64
                    # (reshuffled to [128, 8]), back to a bf16 [1, NCHUNK] row.
                    oraw = small.tile([65, NCHUNK], F32, tag="oraw", name="oraw_t")
                    nc.vector.tensor_copy(out=oraw, in_=ot)
                    rcin = small.tile([128, NCHUNK // 128], F32, tag="rcin", name="rcin_t")
                    nc.sync.dma_start(out=rcin, in_=oraw[64:65, :])
                    rc = small.tile([128, NCHUNK // 128], F32, tag="rc", name="rc_t")
                    nc.vector.reciprocal(out=rc, in_=rcin)
                    rcb = small.tile([128, NCHUNK // 128], BF16, tag="rcb", name="rcb_t")
                    nc.vector.tensor_copy(out=rcb, in_=rc)
                    rflat = small.tile([1, NCHUNK], BF16, tag="rflat", name="rflat_t")
                    nc.sync.dma_start(out=rflat, in_=rcb)
                    if it < NCHUNKS * HPG * MS:  # defer part 2 by ~4 iters
                        pending_norm2[it + 3] = (
                            lambda c=chunk, hh=h, o=oraw, r=rflat: norm2(c, hh, o, r)
                        )
                    else:  # last head: handled in the epilogue
                        last_norm2 = lambda o=oraw, r=rflat: norm2(chunk, h, o, r)
            # ---- epilogue: reserved chunk-0 groups keep the PE warm while
            # the last reciprocal chain completes, then chunk 1's projection.
            assert not quanta and not pending_norm2, (len(quanta), len(pending_norm2))
            for msl, j in tail_reserve:
                emit_f(0, msl, j)
            last_norm2()
            for msl in range(D // 128):
                for j in range(0, NCHUNK, 512):
                    emit_f(1, msl, j)
    nc.compile()
    return nc


_NC = None


def _get_nc():
    global _NC
    if _NC is None:
        _NC = _build_bass()
    return _NC


def build_in_maps(inputs):
    x = np.asarray(inputs["x"], dtype=np.float32)
    W_Q = np.asarray(inputs["W_Q"], dtype=np.float32)
    W_K = np.asarray(inputs["W_K"], dtype=np.float32)
    W_V = np.asarray(inputs["W_V"], dtype=np.float32)
    W_O = np.asarray(inputs["W_O"], dtype=np.float32)
    b_Q = np.asarray(inputs["b_Q"], dtype=np.float32)
    b_K = np.asarray(inputs["b_K"], dtype=np.float32)
    b_V = np.asarray(inputs["b_V"], dtype=np.float32)

    in_maps = []
    for c in range(NCORES):
        b, g = divmod(c, GROUPS)
        sl = slice(g * CH, (g + 1) * CH)
        wqkv = np.concatenate(
            [W_Q[sl, :].T, W_K[sl, :].T, W_V[sl, :].T], axis=1
        )
        in_maps.append(
            {
                "xT": np.ascontiguousarray(x[b].T.astype(ml_dtypes.bfloat16)),
                "wqkv": np.ascontiguousarray(wqkv.astype(ml_dtypes.bfloat16)),
                "woT": np.ascontiguousarray(W_O[:, sl].T.astype(ml_dtypes.bfloat16)),
                "bq": np.ascontiguousarray(b_Q[sl]),
                "bk": np.ascontiguousarray(b_K[sl]),
                "bv": np.ascontiguousarray(b_V[sl]),
            }
        )
    return in_maps


def kernel(**inputs):
    in_maps = build_in_maps(inputs)
    nc = _get_nc()
    res = run_bass_kernel_spmd(nc, in_maps, core_ids=list(range(NCORES)))

    b_O = np.asarray(inputs["b_O"], dtype=np.float32)
    out = np.zeros((B, N, D), dtype=np.float32)
    for c in range(NCORES):
        b = c // GROUPS
        out[b] += res.results[c]["yT"].T.astype(np.float32)
    out += b_O
    return out
